# revision 3
# baseline (speedup 1.0000x reference)
"""Depthwise 13x13 stride-4 conv (AntiAliasInterpolation2d) on 8 TRN2 NeuronCores.

Pure data parallel: batch 32 -> 4 images per core. Two device graphs:

1. rank-1 path (used when each channel's 13x13 kernel is an outer product
   v ⊗ h, which holds for the Gaussian anti-alias kernel): separable conv.
   Stage V contracts input rows on the TensorEngine via banded-Toeplitz
   stationaries (stride-4 vertical conv, fp32 PSUM accumulate); the DVE
   copies V to SBUF in bf16 while de-interleaving columns into 4 phases;
   stage H applies the horizontal taps as full-128 diagonal-stationary
   matmuls, one per tap, accumulating in PSUM (the stride-4 column gather
   becomes a contiguous slice in phase space). The 13 diag(h_j) [128,128]
   stationaries per channel are built ON DEVICE (identity * per-partition
   h scalar on the otherwise-idle DVE/GPSIMD engines) so the DMA stream
   stays input-dominated. PE cost is ~90ns/matmul instruction-issue-bound,
   so H uses 13 big matmuls per channel, not 52 32-row tile matmuls.

2. general path (fallback for non-separable weights): direct 2D conv as
   52 PSUM-accumulated banded-Toeplitz matmuls per channel (13 kernel
   columns x 4 row chunks), stride-4 columns de-interleaved on the host.

Everything computes in bf16 (fp32 accumulation); output is fp32.
"""

import numpy as np
import ml_dtypes

N_CORES = 8
B, C, H, W = 32, 3, 512, 512
KS = 13          # kernel size
PAD = 6          # pad on each side
STR = 4          # stride
OH = OW = 128    # output spatial
PW = W + 2 * PAD  # 524 padded width
NPH = PW // STR   # 131 columns per phase
BPC = B // N_CORES  # images per core = 4
XW = BPC * PW     # 2096 free-dim columns per input tile

# general path epack layout
SLOT = 130
NPAIR = C * KS
EPACK_COLS = (NPAIR - 1) * SLOT + 224

_CACHE = {}

EAV = 224        # banded-Toeplitz stationary band width (shared by 4 chunks)
STCOLS = EAV + 16  # per-channel st: [av band 224 | h taps 13 (pad 16)]
HD = KS * 128    # per-channel diag stationaries built on device


def _bacc():
    from concourse import bacc

    return bacc.Bacc(
        "TRN2", target_bir_lowering=False, debug=False, num_devices=N_CORES
    )


def _av_slice(base):
    # per-chunk lhsT column ranges for the vertical Toeplitz
    return [(base + 96 - 32 * k, base + 224 - 32 * k) for k in range(4)]


def _build_graph_rank1_raw():
    """Hand-scheduled raw-bacc version: no Tile framework.

    Static buffers: all 3 channels' inputs resident in SBUF (DMAs issued
    back-to-back at t=0), double-buffered V/out staging, 7 PSUM banks
    (4 vertical accumulators + B-strip + 2 horizontal accumulators).
    """
    import concourse.bass as bass  # noqa: F401
    from concourse import mybir
    from contextlib import ExitStack

    nc = _bacc()
    x = nc.dram_tensor("x", [C, 128, 4 * XW], mybir.dt.bfloat16, kind="ExternalInput")
    st = nc.dram_tensor("st", [C, 128, STCOLS], mybir.dt.bfloat16, kind="ExternalInput")
    ident = nc.dram_tensor("ident", [128, 128], mybir.dt.bfloat16, kind="ExternalInput")
    ht = nc.dram_tensor("ht", [C, 128, 16], mybir.dt.float32, kind="ExternalInput")
    out = nc.dram_tensor("out", [BPC, C, OH, OW], mybir.dt.bfloat16, kind="ExternalOutput")

    f32 = mybir.dt.float32
    bf16 = mybir.dt.bfloat16
    CW = 4 * XW  # input elems per channel

    with nc.cleanup_on_exit(), ExitStack() as es:
        xt = es.enter_context(nc.sbuf_tensor("xt", [128, 3 * CW], bf16))
        stt = es.enter_context(nc.sbuf_tensor("stt", [128, 3 * STCOLS], bf16))
        idt = es.enter_context(nc.sbuf_tensor("idt", [128, 128], bf16))
        htt = es.enter_context(nc.sbuf_tensor("htt", [128, C * 16], f32))
        hdt = es.enter_context(nc.sbuf_tensor("hdt", [128, 3 * HD], bf16))
        vsb = es.enter_context(nc.sbuf_tensor("vsb", [128, 2 * XW], bf16))
        ot = es.enter_context(nc.sbuf_tensor("ot", [128, 2 * 512], bf16))
        vA = es.enter_context(nc.psum_tensor("vA", [128, 4 * 512], f32))
        vB = es.enter_context(nc.psum_tensor("vB", [128, 512], f32))
        hp2 = es.enter_context(nc.psum_tensor("hp2", [128, 2 * 512], f32))

        s_x = [
            [es.enter_context(nc.semaphore(f"s_x{c}_{k}")) for k in range(4)]
            for c in range(C)
        ]
        s_st = es.enter_context(nc.semaphore("s_st"))
        s_id = es.enter_context(nc.semaphore("s_id"))
        s_hd = [es.enter_context(nc.semaphore(f"s_hd{c}")) for c in range(C)]
        s_mm = [es.enter_context(nc.semaphore(f"s_mm{c}")) for c in range(C)]
        s_vc = [es.enter_context(nc.semaphore(f"s_vc{c}")) for c in range(C)]
        s_out = [es.enter_context(nc.semaphore(f"s_out{c}")) for c in range(C)]
        s_od = [es.enter_context(nc.semaphore(f"s_od{c}")) for c in range(C)]

        with nc.Block() as block:

            @block.sync
            def _(sync):
                # one ring, consumption order: the SDMA drains these FIFO at
                # ~line rate, pacing the PE's k-waves
                for c in range(C):
                    for k in range(4):
                        sync.dma_start(
                            xt[:, c * CW + k * XW : c * CW + (k + 1) * XW],
                            x[c][:, k * XW : (k + 1) * XW],
                        ).then_inc(s_x[c][k], 16)
                for c in range(C):
                    sync.wait_ge(s_out[c], 1)
                    dst = out[:, c].rearrange("g y x -> y g x")
                    src = ot[:, (c % 2) * 512 : (c % 2) * 512 + 512].rearrange(
                        "y (g xx) -> y g xx", g=BPC
                    )
                    sync.dma_start(dst, src).then_inc(s_od[c], 16)
                for c in range(C):
                    # the final channel's receipt wait is optional: nothing
                    # in the kernel consumes s_od[2], and the host reads the
                    # output ms later, so the postamble can overlap the last
                    # DMA's in-flight HBM receipt (~2.5 us)
                    if c == C - 1:
                        continue
                    sync.wait_ge(s_od[c], 16)

            @block.scalar
            def _(scalar):
                scalar.dma_start(idt[:], ident[:]).then_inc(s_id, 16)
                scalar.dma_start(
                    htt[:].rearrange("p (c j) -> p c j", c=C),
                    ht[:].rearrange("c p j -> p c j"),
                ).then_inc(s_id, 16)
                scalar.dma_start(
                    stt[:].rearrange("p (c j) -> p c j", c=C),
                    st[:].rearrange("c p j -> p c j"),
                ).then_inc(s_st, 16)

            def emit_hd_build(eng, c):
                # diag(h_j) stationaries: identity * per-partition h scalar
                eng.wait_ge(s_id, 32)
                for j in range(KS):
                    op = eng.tensor_scalar_mul(
                        hdt[:, c * HD + j * 128 : c * HD + (j + 1) * 128],
                        idt[:],
                        htt[:, c * 16 + j : c * 16 + j + 1],
                    )
                op.then_inc(s_hd[c], 1)

            @block.gpsimd
            def _(gpsimd):
                emit_hd_build(gpsimd, 1)
                emit_hd_build(gpsimd, 2)

            @block.tensor
            def _(tensor):
                def emit_V_wave(c, k):
                    av0 = c * STCOLS
                    xc0 = c * CW
                    if k == 0:
                        tensor.wait_ge(s_st, 16)
                    tensor.wait_ge(s_x[c][k], 16)
                    lo, hi = _av_slice(av0)[k]
                    lhsT = stt[:, lo:hi]
                    for g in range(BPC):
                        if k == 0 and c >= 1:
                            # cast order g0,g1,B,g2,g3 -> bank g frees at:
                            tensor.wait_ge(s_vc[c - 1], (1, 2, 4, 5)[g])
                        mm = tensor.matmul(
                            vA[:, g * 512 : g * 512 + 512],
                            lhsT,
                            xt[:, xc0 + (k * BPC + g) * PW : xc0 + (k * BPC + g) * PW + 512],
                            start=(k == 0),
                            stop=(k == 3),
                            skip_group_check=True,
                        )
                        if k == 3:
                            mm.then_inc(s_mm[c], 1)

                def emit_B(c):
                    av0 = c * STCOLS
                    xc0 = c * CW
                    if c >= 1:
                        tensor.wait_ge(s_vc[c - 1], 3)  # castB(c-1) done
                    xg = xt[:, xc0 : xc0 + CW].rearrange(
                        "p (k g w) -> p k g w", k=4, g=BPC
                    )
                    for k in range(4):
                        mm = tensor.matmul(
                            vB[:, 0 : BPC * 12],
                            stt[:, _av_slice(av0)[k][0] : _av_slice(av0)[k][1]],
                            xg[:, k, :, 512:524],
                            start=(k == 0),
                            stop=(k == 3),
                            skip_group_check=True,
                        )
                    mm.then_inc(s_mm[c], 1)

                def emit_H(c):
                    b0 = (c % 2) * XW
                    h0 = (c % 2) * 512
                    tensor.wait_ge(s_vc[c], 5)
                    tensor.wait_ge(s_hd[c], 1)
                    if c >= 2:
                        tensor.wait_ge(s_out[c - 2], 1)  # hp bank WAR
                    vg = vsb[:, b0 : b0 + XW].rearrange("p (g w) -> p g w", g=BPC)
                    for j in range(KS):
                        ph, q = j % STR, j // STR
                        off = ph * NPH + q
                        mm = tensor.matmul(
                            hp2[:, h0 : h0 + 512],
                            hdt[:, c * HD + j * 128 : c * HD + (j + 1) * 128],
                            vg[:, :, off : off + OW],
                            start=(j == 0),
                            stop=(j == KS - 1),
                            skip_group_check=True,
                        )
                    mm.then_inc(s_mm[c], 1)

                # channel-block pipeline: VB0 VB1 H0 VB2 H1 H2
                for k in range(4):
                    emit_V_wave(0, k)
                emit_B(0)
                for k in range(4):
                    emit_V_wave(1, k)
                emit_B(1)
                emit_H(0)
                for k in range(4):
                    emit_V_wave(2, k)
                emit_B(2)
                emit_H(1)
                emit_H(2)

            @block.vector
            def _(vector):
                emit_hd_build(vector, 0)

                def emit_casts(c):
                    # order: g0, g1, castB, g2, g3 -> s_vc counts 1,2,3,4,5.
                    # castB lands right after the vB matmuls, so the first
                    # image pair plus its B-strip (count 3) unblocks early.
                    b0 = (c % 2) * XW
                    vg = vsb[:, b0 : b0 + XW].rearrange(
                        "p (g ph u) -> p g ph u", g=BPC, ph=STR
                    )

                    def castA(g):
                        vector.wait_ge(s_mm[c], g + 1)
                        if c >= 2 and g == 0:
                            vector.wait_ge(s_mm[c - 2], 6)  # vsb WAR vs H(c-2)
                        srcA = vA[:, g * 512 : g * 512 + 512].rearrange(
                            "p (u ph) -> p ph u", ph=STR
                        )
                        vector.tensor_copy(vg[:, g, :, 0:128], srcA).then_inc(
                            s_vc[c], 1
                        )

                    castA(0)
                    castA(1)
                    vector.wait_ge(s_mm[c], 5)
                    srcB = vB[:, 0 : BPC * 12].rearrange(
                        "p (g u ph) -> p g ph u", g=BPC, ph=STR
                    )
                    vector.tensor_copy(vg[:, :, :, 128:131], srcB).then_inc(
                        s_vc[c], 1
                    )
                    castA(2)
                    castA(3)

                def emit_out(c):
                    h0 = (c % 2) * 512
                    vector.wait_ge(s_mm[c], 6)
                    if c >= 2:
                        vector.wait_ge(s_od[c - 2], 16)  # ot slot WAR
                    vector.tensor_copy(
                        ot[:, (c % 2) * 512 : (c % 2) * 512 + 512],
                        hp2[:, h0 : h0 + 512],
                    ).then_inc(s_out[c], 1)

                # mirror the PE pipeline: VB0 VB1 H0 VB2 H1 H2
                emit_casts(0)
                emit_casts(1)
                emit_out(0)
                emit_casts(2)
                emit_out(1)
                emit_out(2)

        nc.all_engine_barrier()
    nc.finalize()
    return nc


def _build_graph_general():
    import concourse.tile as tile
    from concourse import mybir

    nc = _bacc()
    x = nc.dram_tensor("x", [C, 4, 128, XW], mybir.dt.bfloat16, kind="ExternalInput")
    ep = nc.dram_tensor("ep", [128, EPACK_COLS], mybir.dt.bfloat16, kind="ExternalInput")
    out = nc.dram_tensor("out", [BPC, C, OH, OW], mybir.dt.float32, kind="ExternalOutput")

    with tile.TileContext(nc) as tc:
        with (
            tc.tile_pool(name="const", bufs=1) as constp,
            tc.tile_pool(name="xin", bufs=4) as xin,
            tc.tile_pool(name="ps", bufs=2, space="PSUM") as psp,
            tc.tile_pool(name="ot", bufs=2) as otp,
        ):
            ept = constp.tile([128, EPACK_COLS], mybir.dt.bfloat16)
            nc.scalar.dma_start(ept[:], ep[:])
            for c in range(C):
                psum = psp.tile([128, BPC * OW], mybir.dt.float32)
                for k in range(4):
                    xt = xin.tile([128, XW], mybir.dt.bfloat16)
                    nc.sync.dma_start(xt[:], x[c, k])
                    xg = xt[:].rearrange("p (g w) -> p g w", g=BPC)
                    for j in range(KS):
                        ph, q = j % STR, j // STR
                        off = ph * NPH + q
                        rhs = xg[:, :, off : off + OW]
                        t = c * KS + j
                        lo = t * SLOT + 96 - 32 * k
                        lhsT = ept[:, lo : lo + 128]
                        nc.tensor.matmul(
                            psum[:],
                            lhsT,
                            rhs,
                            start=(k == 0 and j == 0),
                            stop=(k == 3 and j == KS - 1),
                        )
                o = otp.tile([128, BPC * OW], mybir.dt.float32)
                nc.vector.tensor_copy(o[:], psum[:])
                dst = out[:, c].rearrange("g y x -> y g x")
                nc.sync.dma_start(dst, o[:].rearrange("y (g x) -> y g x", g=BPC))
    nc.finalize()
    return nc


def _decompose(weight):
    """Per-channel SVD; return (v[c,13], h[c,13]) if rank-1, else None."""
    vs, hs = [], []
    for c in range(C):
        w = weight[c, 0].astype(np.float64)
        u, s, vt = np.linalg.svd(w)
        if s[1] > 1e-5 * s[0]:
            return None
        sc = np.sqrt(s[0])
        vs.append(u[:, 0] * sc)
        hs.append(vt[0] * sc)
    return np.stack(vs), np.stack(hs)


def _pad_shard(inp):
    """[32,3,512,512] f32 -> [core, c, 128, k*img*524] bf16 (padded cols)."""
    bf16 = ml_dtypes.bfloat16
    pad = np.zeros((B, C, H, PW), np.float32)
    pad[..., PAD : PAD + W] = inp
    arr = pad.reshape(N_CORES, BPC, C, 4, 128, PW)
    arr = arr.transpose(0, 2, 4, 3, 1, 5).reshape(N_CORES, C, 128, 4 * XW)
    return np.ascontiguousarray(arr).astype(bf16)


def _phase_shard(inp):
    """[32,3,512,512] f32 -> padded + phase-deinterleaved shards (general)."""
    bf16 = ml_dtypes.bfloat16
    pad = np.zeros((B, C, H, PW), np.float32)
    pad[..., PAD : PAD + W] = inp
    phmat = pad.reshape(B, C, H, NPH, STR).transpose(0, 1, 2, 4, 3)
    arr = phmat.reshape(N_CORES, BPC, C, 4, 128, STR, NPH)
    arr = arr.transpose(0, 2, 3, 4, 1, 5, 6).reshape(N_CORES, C, 4, 128, XW)
    return np.ascontiguousarray(arr).astype(bf16)


def _prep_rank1(inp, v, h):
    bf16 = ml_dtypes.bfloat16
    arr = _pad_shard(inp)
    st = np.zeros((C, 128, STCOLS), np.float32)
    rr = np.arange(128)[:, None]
    cc = np.arange(EAV)[None, :]
    taps = rr - 4 * (cc - 96) + PAD  # E[r, col] = v[tap] (EBAND layout)
    mband = (taps >= 0) & (taps < KS)
    for c in range(C):
        E = np.zeros((128, EAV), np.float32)
        E[mband] = v[c][taps[mband]]
        st[c, :, :EAV] = E
    st = st.astype(bf16)
    ident = np.eye(128, dtype=np.float32).astype(bf16)
    # h taps replicated down partitions (f32 per-partition scalars for the
    # on-device diag build; bf16-round first so device matches host math)
    ht = np.zeros((C, 128, 16), np.float32)
    ht[:, :, :KS] = h.astype(bf16).astype(np.float32)[:, None, :]
    return [
        {"x": arr[core], "st": st, "ident": ident, "ht": ht}
        for core in range(N_CORES)
    ]


def _prep_general(inp, weight):
    bf16 = ml_dtypes.bfloat16
    arr = _phase_shard(inp)
    epk = np.zeros((128, EPACK_COLS), np.float32)
    r = np.arange(128)
    for c in range(C):
        for j in range(KS):
            t = c * KS + j
            for s in range(-2, 34):
                i = r - 4 * s + PAD
                m = (i >= 0) & (i < KS)
                if m.any():
                    epk[m, t * SLOT + 96 + s] = weight[c, 0, i[m], j]
    epk = epk.astype(bf16)
    return [{"x": arr[core], "ep": epk} for core in range(N_CORES)]


def _prep(inp, weight):
    """Returns (graph_key, in_maps)."""
    inp = np.asarray(inp, dtype=np.float32)
    weight = np.asarray(weight, dtype=np.float32)
    vh = _decompose(weight)
    if vh is not None:
        return "rank1", _prep_rank1(inp, *vh)
    return "general", _prep_general(inp, weight)


_BUILDERS = {
    "rank1": lambda: _build_graph_rank1_raw(),
    "general": lambda: _build_graph_general(),
}


def _graph(key):
    if key not in _CACHE:
        _CACHE[key] = _BUILDERS[key]()
    return _CACHE[key]


def _run(key, in_maps):
    from concourse.bass_utils import run_bass_kernel_spmd

    nc = _graph(key)
    res = run_bass_kernel_spmd(nc, in_maps, core_ids=list(range(N_CORES)))
    outs = [res.results[i]["out"] for i in range(N_CORES)]
    return np.concatenate(outs, axis=0).astype(np.float32)


def kernel(inp, weight):
    inp = np.asarray(inp, dtype=np.float32)
    weight = np.asarray(weight, dtype=np.float32)
    key, in_maps = _prep(inp, weight)
    try:
        return _run(key, in_maps)
    except Exception:
        if key == "general":
            raise
        # fall back to the general (weight-agnostic) graph
        return _run("general", _prep_general(inp, weight))


# revision 4
# speedup vs baseline: 1.8357x; 1.8357x over previous
"""Depthwise 13x13 stride-4 conv (AntiAliasInterpolation2d) on 8 TRN2 NeuronCores.

Pure data parallel: batch 32 -> 4 images per core. Two device graphs:

1. rank-1 path (used when each channel's 13x13 kernel is an outer product
   v ⊗ h, which holds for the Gaussian anti-alias kernel): separable conv.
   Stage V contracts input rows on the TensorEngine via banded-Toeplitz
   stationaries (stride-4 vertical conv, fp32 PSUM accumulate); the DVE
   copies V to SBUF in bf16 while de-interleaving columns into 4 phases;
   stage H applies the horizontal taps as full-128 diagonal-stationary
   matmuls, one per tap, accumulating in PSUM (the stride-4 column gather
   becomes a contiguous slice in phase space). The 13 diag(h_j) [128,128]
   stationaries per channel stream from HBM just-in-time, slotted into
   the input DMA ring in consumption order. The PE is stream-bound
   (~1 moving column/cycle); H is 13 full-width matmuls per channel.

2. general path (fallback for non-separable weights): direct 2D conv as
   52 PSUM-accumulated banded-Toeplitz matmuls per channel (13 kernel
   columns x 4 row chunks), stride-4 columns de-interleaved on the host.

Everything computes in bf16 (fp32 accumulation); output is fp32.
"""

import numpy as np
import ml_dtypes

N_CORES = 8
B, C, H, W = 32, 3, 512, 512
KS = 13          # kernel size
PAD = 6          # pad on each side
STR = 4          # stride
OH = OW = 128    # output spatial
PW = W + 2 * PAD  # 524 padded width
NPH = PW // STR   # 131 columns per phase
BPC = B // N_CORES  # images per core = 4
XW = BPC * PW     # 2096 free-dim columns per input tile

# general path epack layout
SLOT = 130
NPAIR = C * KS
EPACK_COLS = (NPAIR - 1) * SLOT + 224

_CACHE = {}

EAV = 224        # banded-Toeplitz stationary band width (shared by 4 chunks)
STCOLS = EAV + 16  # per-channel st: [av band 224 | h taps 13 (pad 16)]
HD = KS * 128    # per-channel diag stationaries built on device


def _bacc():
    from concourse import bacc

    return bacc.Bacc(
        "TRN2", target_bir_lowering=False, debug=False, num_devices=N_CORES
    )


def _av_slice(base):
    # per-chunk lhsT column ranges for the vertical Toeplitz
    return [(base + 96 - 32 * k, base + 224 - 32 * k) for k in range(4)]


def _build_graph_rank1_raw():
    """Hand-scheduled raw-bacc version: no Tile framework.

    Static buffers: all 3 channels' inputs resident in SBUF (DMAs issued
    back-to-back at t=0), double-buffered V/out staging, 7 PSUM banks
    (4 vertical accumulators + B-strip + 2 horizontal accumulators).
    """
    import concourse.bass as bass  # noqa: F401
    from concourse import mybir
    from contextlib import ExitStack

    nc = _bacc()
    x = nc.dram_tensor("x", [C, 128, 4 * XW], mybir.dt.bfloat16, kind="ExternalInput")
    st = nc.dram_tensor("st", [C, 128, STCOLS], mybir.dt.bfloat16, kind="ExternalInput")
    hd = nc.dram_tensor("hd", [C, 128, HD], mybir.dt.bfloat16, kind="ExternalInput")
    out = nc.dram_tensor("out", [BPC, C, OH, OW], mybir.dt.bfloat16, kind="ExternalOutput")

    f32 = mybir.dt.float32
    bf16 = mybir.dt.bfloat16
    CW = 4 * XW  # input elems per channel

    with nc.cleanup_on_exit(), ExitStack() as es:
        xt = es.enter_context(nc.sbuf_tensor("xt", [128, 3 * CW], bf16))
        stt = es.enter_context(nc.sbuf_tensor("stt", [128, 3 * STCOLS], bf16))
        hdt = es.enter_context(nc.sbuf_tensor("hdt", [128, 3 * HD], bf16))
        vsb = es.enter_context(nc.sbuf_tensor("vsb", [128, 2 * XW], bf16))
        ot = es.enter_context(nc.sbuf_tensor("ot", [128, 2 * 512], bf16))
        vA = es.enter_context(nc.psum_tensor("vA", [128, 4 * 512], f32))
        vB = es.enter_context(nc.psum_tensor("vB", [128, 512], f32))
        hp2 = es.enter_context(nc.psum_tensor("hp2", [128, 2 * 512], f32))

        s_x = [
            [es.enter_context(nc.semaphore(f"s_x{c}_{k}")) for k in range(4)]
            for c in range(C)
        ]
        s_st = es.enter_context(nc.semaphore("s_st"))
        s_hd = [es.enter_context(nc.semaphore(f"s_hd{c}")) for c in range(C)]
        s_mm = [es.enter_context(nc.semaphore(f"s_mm{c}")) for c in range(C)]
        s_vc = [es.enter_context(nc.semaphore(f"s_vc{c}")) for c in range(C)]
        s_out = [es.enter_context(nc.semaphore(f"s_out{c}")) for c in range(C)]
        s_od = [es.enter_context(nc.semaphore(f"s_od{c}")) for c in range(C)]

        with nc.Block() as block:

            @block.sync
            def _(sync):
                # one ring, consumption order: the SDMA drains these FIFO
                # at ~line rate, pacing the PE's k-waves. Diag stationaries
                # hd[c] are slotted just-in-time: after x[c] (they are needed
                # ~1us after x[c] fully lands), except hd[2] which must not
                # trail the final input chunk.
                def emit_x(c):
                    for k in range(4):
                        sync.dma_start(
                            xt[:, c * CW + k * XW : c * CW + (k + 1) * XW],
                            x[c][:, k * XW : (k + 1) * XW],
                        ).then_inc(s_x[c][k], 16)

                def emit_hd(c):
                    sync.dma_start(
                        hdt[:, c * HD : (c + 1) * HD], hd[c]
                    ).then_inc(s_hd[c], 16)

                emit_x(0)
                emit_hd(0)
                emit_x(1)
                emit_hd(1)
                emit_hd(2)
                emit_x(2)
                for c in range(C):
                    sync.wait_ge(s_out[c], 1)
                    dst = out[:, c].rearrange("g y x -> y g x")
                    src = ot[:, (c % 2) * 512 : (c % 2) * 512 + 512].rearrange(
                        "y (g xx) -> y g xx", g=BPC
                    )
                    sync.dma_start(dst, src).then_inc(s_od[c], 16)
                for c in range(C):
                    # the final channel's receipt wait is optional: nothing
                    # in the kernel consumes s_od[2], and the host reads the
                    # output ms later, so the postamble can overlap the last
                    # DMA's in-flight HBM receipt (~2.5 us)
                    if c == C - 1:
                        continue
                    sync.wait_ge(s_od[c], 16)

            @block.scalar
            def _(scalar):
                scalar.dma_start(
                    stt[:].rearrange("p (c j) -> p c j", c=C),
                    st[:].rearrange("c p j -> p c j"),
                ).then_inc(s_st, 16)

            @block.tensor
            def _(tensor):
                def emit_V_wave(c, k):
                    av0 = c * STCOLS
                    xc0 = c * CW
                    if k == 0:
                        tensor.wait_ge(s_st, 16)
                    tensor.wait_ge(s_x[c][k], 16)
                    lo, hi = _av_slice(av0)[k]
                    lhsT = stt[:, lo:hi]
                    for g in range(BPC):
                        if k == 0 and c >= 1:
                            # cast order g0,g1,B,g2,g3 -> bank g frees at:
                            tensor.wait_ge(s_vc[c - 1], (1, 2, 4, 5)[g])
                        mm = tensor.matmul(
                            vA[:, g * 512 : g * 512 + 512],
                            lhsT,
                            xt[:, xc0 + (k * BPC + g) * PW : xc0 + (k * BPC + g) * PW + 512],
                            start=(k == 0),
                            stop=(k == 3),
                            skip_group_check=True,
                        )
                        if k == 3:
                            mm.then_inc(s_mm[c], 1)

                def emit_B(c):
                    av0 = c * STCOLS
                    xc0 = c * CW
                    if c >= 1:
                        tensor.wait_ge(s_vc[c - 1], 3)  # castB(c-1) done
                    xg = xt[:, xc0 : xc0 + CW].rearrange(
                        "p (k g w) -> p k g w", k=4, g=BPC
                    )
                    for k in range(4):
                        mm = tensor.matmul(
                            vB[:, 0 : BPC * 12],
                            stt[:, _av_slice(av0)[k][0] : _av_slice(av0)[k][1]],
                            xg[:, k, :, 512:524],
                            start=(k == 0),
                            stop=(k == 3),
                            skip_group_check=True,
                        )
                    mm.then_inc(s_mm[c], 1)

                def emit_H(c, glo, ghi, gate, war=False):
                    b0 = (c % 2) * XW
                    h0 = (c % 2) * 512
                    tensor.wait_ge(s_vc[c], gate)
                    tensor.wait_ge(s_hd[c], 16)
                    if war and c >= 2:
                        tensor.wait_ge(s_out[c - 2], 1)  # hp bank WAR
                    vg = vsb[:, b0 : b0 + XW].rearrange("p (g w) -> p g w", g=BPC)
                    for j in range(KS):
                        ph, q = j % STR, j // STR
                        off = ph * NPH + q
                        mm = tensor.matmul(
                            hp2[:, h0 + glo * OW : h0 + ghi * OW],
                            hdt[:, c * HD + j * 128 : c * HD + (j + 1) * 128],
                            vg[:, glo:ghi, off : off + OW],
                            start=(j == 0),
                            stop=(j == KS - 1),
                            skip_group_check=True,
                        )
                    mm.then_inc(s_mm[c], 1)

                # channel-block pipeline: VB0 VB1 H0 VB2 H1 H2; the last
                # channel's H runs after the final input chunk, so it goes in
                # two image-pair groups, the first gated only on casts g0,g1+B
                for k in range(4):
                    emit_V_wave(0, k)
                emit_B(0)
                for k in range(4):
                    emit_V_wave(1, k)
                emit_B(1)
                emit_H(0, 0, BPC, 5)
                for k in range(4):
                    emit_V_wave(2, k)
                emit_B(2)
                emit_H(1, 0, BPC, 5)
                emit_H(2, 0, 2, 3, war=True)
                emit_H(2, 2, BPC, 5)

            @block.vector
            def _(vector):
                def emit_casts(c):
                    # order: g0, g1, castB, g2, g3 -> s_vc counts 1,2,3,4,5.
                    # castB lands right after the vB matmuls, so the first
                    # image pair plus its B-strip (count 3) unblocks early.
                    b0 = (c % 2) * XW
                    vg = vsb[:, b0 : b0 + XW].rearrange(
                        "p (g ph u) -> p g ph u", g=BPC, ph=STR
                    )

                    def castA(g):
                        vector.wait_ge(s_mm[c], g + 1)
                        if c >= 2 and g == 0:
                            vector.wait_ge(s_mm[c - 2], 6)  # vsb WAR vs H(c-2)
                        srcA = vA[:, g * 512 : g * 512 + 512].rearrange(
                            "p (u ph) -> p ph u", ph=STR
                        )
                        vector.tensor_copy(vg[:, g, :, 0:128], srcA).then_inc(
                            s_vc[c], 1
                        )

                    castA(0)
                    castA(1)
                    vector.wait_ge(s_mm[c], 5)
                    srcB = vB[:, 0 : BPC * 12].rearrange(
                        "p (g u ph) -> p g ph u", g=BPC, ph=STR
                    )
                    vector.tensor_copy(vg[:, :, :, 128:131], srcB).then_inc(
                        s_vc[c], 1
                    )
                    castA(2)
                    castA(3)

                def emit_out(c):
                    h0 = (c % 2) * 512
                    vector.wait_ge(s_mm[c], 7 if c == 2 else 6)
                    if c >= 2:
                        vector.wait_ge(s_od[c - 2], 16)  # ot slot WAR
                    vector.tensor_copy(
                        ot[:, (c % 2) * 512 : (c % 2) * 512 + 512],
                        hp2[:, h0 : h0 + 512],
                    ).then_inc(s_out[c], 1)

                # mirror the PE pipeline: VB0 VB1 H0 VB2 H1 H2
                emit_casts(0)
                emit_casts(1)
                emit_out(0)
                emit_casts(2)
                emit_out(1)
                emit_out(2)

        nc.all_engine_barrier()
    nc.finalize()
    return nc


def _build_graph_general():
    import concourse.tile as tile
    from concourse import mybir

    nc = _bacc()
    x = nc.dram_tensor("x", [C, 4, 128, XW], mybir.dt.bfloat16, kind="ExternalInput")
    ep = nc.dram_tensor("ep", [128, EPACK_COLS], mybir.dt.bfloat16, kind="ExternalInput")
    out = nc.dram_tensor("out", [BPC, C, OH, OW], mybir.dt.float32, kind="ExternalOutput")

    with tile.TileContext(nc) as tc:
        with (
            tc.tile_pool(name="const", bufs=1) as constp,
            tc.tile_pool(name="xin", bufs=4) as xin,
            tc.tile_pool(name="ps", bufs=2, space="PSUM") as psp,
            tc.tile_pool(name="ot", bufs=2) as otp,
        ):
            ept = constp.tile([128, EPACK_COLS], mybir.dt.bfloat16)
            nc.scalar.dma_start(ept[:], ep[:])
            for c in range(C):
                psum = psp.tile([128, BPC * OW], mybir.dt.float32)
                for k in range(4):
                    xt = xin.tile([128, XW], mybir.dt.bfloat16)
                    nc.sync.dma_start(xt[:], x[c, k])
                    xg = xt[:].rearrange("p (g w) -> p g w", g=BPC)
                    for j in range(KS):
                        ph, q = j % STR, j // STR
                        off = ph * NPH + q
                        rhs = xg[:, :, off : off + OW]
                        t = c * KS + j
                        lo = t * SLOT + 96 - 32 * k
                        lhsT = ept[:, lo : lo + 128]
                        nc.tensor.matmul(
                            psum[:],
                            lhsT,
                            rhs,
                            start=(k == 0 and j == 0),
                            stop=(k == 3 and j == KS - 1),
                        )
                o = otp.tile([128, BPC * OW], mybir.dt.float32)
                nc.vector.tensor_copy(o[:], psum[:])
                dst = out[:, c].rearrange("g y x -> y g x")
                nc.sync.dma_start(dst, o[:].rearrange("y (g x) -> y g x", g=BPC))
    nc.finalize()
    return nc


def _decompose(weight):
    """Per-channel SVD; return (v[c,13], h[c,13]) if rank-1, else None."""
    vs, hs = [], []
    for c in range(C):
        w = weight[c, 0].astype(np.float64)
        u, s, vt = np.linalg.svd(w)
        if s[1] > 1e-5 * s[0]:
            return None
        sc = np.sqrt(s[0])
        vs.append(u[:, 0] * sc)
        hs.append(vt[0] * sc)
    return np.stack(vs), np.stack(hs)


def _pad_shard(inp):
    """[32,3,512,512] f32 -> [core, c, 128, k*img*524] bf16 (padded cols)."""
    bf16 = ml_dtypes.bfloat16
    pad = np.zeros((B, C, H, PW), np.float32)
    pad[..., PAD : PAD + W] = inp
    arr = pad.reshape(N_CORES, BPC, C, 4, 128, PW)
    arr = arr.transpose(0, 2, 4, 3, 1, 5).reshape(N_CORES, C, 128, 4 * XW)
    return np.ascontiguousarray(arr).astype(bf16)


def _phase_shard(inp):
    """[32,3,512,512] f32 -> padded + phase-deinterleaved shards (general)."""
    bf16 = ml_dtypes.bfloat16
    pad = np.zeros((B, C, H, PW), np.float32)
    pad[..., PAD : PAD + W] = inp
    phmat = pad.reshape(B, C, H, NPH, STR).transpose(0, 1, 2, 4, 3)
    arr = phmat.reshape(N_CORES, BPC, C, 4, 128, STR, NPH)
    arr = arr.transpose(0, 2, 3, 4, 1, 5, 6).reshape(N_CORES, C, 4, 128, XW)
    return np.ascontiguousarray(arr).astype(bf16)


def _prep_rank1(inp, v, h):
    bf16 = ml_dtypes.bfloat16
    arr = _pad_shard(inp)
    st = np.zeros((C, 128, STCOLS), np.float32)
    rr = np.arange(128)[:, None]
    cc = np.arange(EAV)[None, :]
    taps = rr - 4 * (cc - 96) + PAD  # E[r, col] = v[tap] (EBAND layout)
    mband = (taps >= 0) & (taps < KS)
    for c in range(C):
        E = np.zeros((128, EAV), np.float32)
        E[mband] = v[c][taps[mband]]
        st[c, :, :EAV] = E
    st = st.astype(bf16)
    # full-width diagonal H stationaries: hd[c, :, j*128 + i] = h_j at i==row
    hdm = np.zeros((C, 128, HD), np.float32)
    idx = np.arange(128)
    for c in range(C):
        for j in range(KS):
            hdm[c, idx, j * 128 + idx] = h[c, j]
    hdm = hdm.astype(bf16)
    return [
        {"x": arr[core], "st": st, "hd": hdm} for core in range(N_CORES)
    ]


def _prep_general(inp, weight):
    bf16 = ml_dtypes.bfloat16
    arr = _phase_shard(inp)
    epk = np.zeros((128, EPACK_COLS), np.float32)
    r = np.arange(128)
    for c in range(C):
        for j in range(KS):
            t = c * KS + j
            for s in range(-2, 34):
                i = r - 4 * s + PAD
                m = (i >= 0) & (i < KS)
                if m.any():
                    epk[m, t * SLOT + 96 + s] = weight[c, 0, i[m], j]
    epk = epk.astype(bf16)
    return [{"x": arr[core], "ep": epk} for core in range(N_CORES)]


def _prep(inp, weight):
    """Returns (graph_key, in_maps)."""
    inp = np.asarray(inp, dtype=np.float32)
    weight = np.asarray(weight, dtype=np.float32)
    vh = _decompose(weight)
    if vh is not None:
        return "rank1", _prep_rank1(inp, *vh)
    return "general", _prep_general(inp, weight)


_BUILDERS = {
    "rank1": lambda: _build_graph_rank1_raw(),
    "general": lambda: _build_graph_general(),
}


def _graph(key):
    if key not in _CACHE:
        _CACHE[key] = _BUILDERS[key]()
    return _CACHE[key]


def _run(key, in_maps):
    from concourse.bass_utils import run_bass_kernel_spmd

    nc = _graph(key)
    res = run_bass_kernel_spmd(nc, in_maps, core_ids=list(range(N_CORES)))
    outs = [res.results[i]["out"] for i in range(N_CORES)]
    return np.concatenate(outs, axis=0).astype(np.float32)


def kernel(inp, weight):
    inp = np.asarray(inp, dtype=np.float32)
    weight = np.asarray(weight, dtype=np.float32)
    key, in_maps = _prep(inp, weight)
    try:
        return _run(key, in_maps)
    except Exception:
        if key == "general":
            raise
        # fall back to the general (weight-agnostic) graph
        return _run("general", _prep_general(inp, weight))
